# revision 14
# baseline (speedup 1.0000x reference)
"""Trainium2 Bass kernel for y = x @ W^T + b  (4096x4096 @ 4096x4096 + 4096).

Sharding: 2D grid, R=4 batch-groups x C=2 out-feature-groups. Core (r, c)
computes yT_rc = W_c @ x_r^T + b_c[:, None]  ([2048, 1024], output
transposed) and the host reassembles y. No collectives.

All layout work happens on the host: x and W slices are transposed,
tiled to the exact SBUF layout, and cast to bf16 in numpy. The device
kernel is nothing but back-to-back bf16 matmuls (fp32 PSUM accumulate):

  - xT_r [128, 32*1024] bf16 resident in SBUF, DMA'd in graduated
    chunks (small first so compute starts early).
  - Prologue: the first 2 o-tiles' accumulations run chunk-major over
    4 PSUM banks so the PE saturates while x is still arriving.
  - Steady state per o-tile: W slab [128, 32*128] bf16 DMA (triple-
    buffered), 32 k-tiles x 2 batch-chunk matmuls (N=512) accumulating
    in PSUM, ScalarE eviction fused with bias add, HWDGE DMA out.

PE roofline: 1024 MM x 512 cols / 2.4 GHz = 218.5 us per core.
"""

import os
import sys

for _p in ("/opt/trn_rl_repo", "/opt/pypackages"):
    if _p not in sys.path and os.path.isdir(_p):
        sys.path.append(_p)

import numpy as np
import ml_dtypes

import concourse.bass as bass
import concourse.tile as tile
from concourse import bacc, mybir
from concourse.bass_utils import run_bass_kernel_spmd

N_CORES = 8
R = 4                          # batch groups
C = 2                          # out-feature groups
BATCH = 4096
IN_F = 4096
OUT_F = 4096
P = 128
BR = BATCH // R                # 1024 batch rows per core
OC = OUT_F // C                # 2048 out features per core
KT = IN_F // P                 # 32 contraction tiles
OT = OC // P                   # 16 output-feature tiles per core
NB = BR // 512                 # 2 psum-width batch chunks
# x chunk schedule: (start_it, n_its); first chunks small so the PE can
# start sooner, later chunks 1MB for DMA efficiency.
XCHUNKS = [(0, 1), (1, 1), (2, 2), (4, 4), (8, 4), (12, 4), (16, 4),
           (20, 4), (24, 4), (28, 4)]
NPRE = 4                       # o-tiles interleaved in the prologue
WSPLIT = 8                     # k-tiles in the first piece of prologue W slabs

_F32 = mybir.dt.float32
_BF16 = mybir.dt.bfloat16
_BF16_NP = ml_dtypes.bfloat16

_compiled_nc = None


def _build():
    nc = bacc.Bacc("TRN2", target_bir_lowering=False, debug=False,
                   num_devices=N_CORES)

    # Host-pretiled layouts (see _prep_inputs):
    #   xt[p, it*BR + b]            = x_r[b, it*128 + p]              (bf16)
    #   wt[ot*128 + p, it*128 + o2] = w_c[ot*128 + o2, it*128 + p]    (bf16)
    #   bias_t[p, ot]               = b_c[ot*128 + p]                 (f32)
    xt = nc.dram_tensor("xt", [P, KT * BR], _BF16, kind="ExternalInput")
    wt = nc.dram_tensor("wt", [OT * P, KT * P], _BF16, kind="ExternalInput")
    bias = nc.dram_tensor("bias", [P, OT], _F32, kind="ExternalInput")
    out = nc.dram_tensor("out", [OC, BR], _F32, kind="ExternalOutput")

    with tile.TileContext(nc) as tc:
        with tc.tile_pool(name="const", bufs=1) as const, \
             tc.tile_pool(name="wslab", bufs=6) as wpool, \
             tc.tile_pool(name="psum", bufs=8, space="PSUM") as pspool, \
             tc.tile_pool(name="yout", bufs=4) as ypool:

            bias_sb = const.tile([P, OT], _F32)
            nc.scalar.dma_start(out=bias_sb[:], in_=bias[:, :])

            # ---- first x chunk on the fast HWDGE ring, ahead of W0a
            x_sb = const.tile([P, KT * BR], _BF16)
            it0, nit = XCHUNKS[0]
            nc.sync.dma_start(out=x_sb[:, it0 * BR:(it0 + nit) * BR],
                              in_=xt[:, it0 * BR:(it0 + nit) * BR])

            # ---- W slabs for the prologue o-tiles, split so only a small
            # first piece gates the first matmuls
            w_pre = []
            for ot in range(NPRE):
                w_sb = wpool.tile([P, KT * P], _BF16, name=f"w{ot}", tag="w")
                nc.sync.dma_start(out=w_sb[:, :WSPLIT * P],
                                  in_=wt[ot * P:(ot + 1) * P, :WSPLIT * P])
                w_pre.append(w_sb)
            for ot in range(NPRE):
                nc.sync.dma_start(out=w_pre[ot][:, WSPLIT * P:],
                                  in_=wt[ot * P:(ot + 1) * P, WSPLIT * P:])

            # ---- rest of x on the gpsimd (SWDGE) ring
            for it0, nit in XCHUNKS[1:]:
                nc.gpsimd.dma_start(
                    out=x_sb[:, it0 * BR:(it0 + nit) * BR],
                    in_=xt[:, it0 * BR:(it0 + nit) * BR])

            def evict(ot, y_sb, ps, j):
                c0 = j * 512
                if j == 0:
                    nc.scalar.activation(
                        y_sb[:, c0:c0 + 512], ps[:],
                        mybir.ActivationFunctionType.Identity,
                        bias=bias_sb[:, ot:ot + 1])
                else:
                    nc.vector.tensor_scalar_add(
                        y_sb[:, c0:c0 + 512], ps[:], bias_sb[:, ot:ot + 1])
                nc.scalar.dma_start(
                    out=out[ot * P:(ot + 1) * P, c0:c0 + 512],
                    in_=y_sb[:, c0:c0 + 512])

            # ---- prologue: first NPRE o-tiles chunk-major over x arrival
            ps_pre = [[pspool.tile([P, 512], _F32, name=f"psp{ot}_{j}",
                                   tag="ps")
                       for j in range(NB)] for ot in range(NPRE)]
            for it0, nit in XCHUNKS:
                for ot in range(NPRE):
                    for l in range(nit):
                        it = it0 + l
                        for j in range(NB):
                            b0 = it * BR + j * 512
                            nc.tensor.matmul(
                                ps_pre[ot][j][:],
                                lhsT=w_pre[ot][:, it * P:(it + 1) * P],
                                rhs=x_sb[:, b0:b0 + 512],
                                start=(it == 0), stop=(it == KT - 1))
            for ot in range(NPRE):
                y_sb = ypool.tile([P, BR], _F32, name=f"y{ot}", tag="y")
                for j in range(NB):
                    evict(ot, y_sb, ps_pre[ot][j], j)

            # ---- steady state over the remaining o-tiles
            for ot in range(NPRE, OT):
                w_sb = wpool.tile([P, KT * P], _BF16, name=f"w{ot}", tag="w")
                nc.sync.dma_start(out=w_sb[:],
                                  in_=wt[ot * P:(ot + 1) * P, :])

                y_sb = ypool.tile([P, BR], _F32, name=f"y{ot}", tag="y")
                for j in range(NB):
                    ps = pspool.tile([P, 512], _F32, name=f"ps{ot}_{j}",
                                     tag="ps")
                    for it in range(KT):
                        b0 = it * BR + j * 512
                        nc.tensor.matmul(ps[:],
                                         lhsT=w_sb[:, it * P:(it + 1) * P],
                                         rhs=x_sb[:, b0:b0 + 512],
                                         start=(it == 0), stop=(it == KT - 1))
                    evict(ot, y_sb, ps, j)

    nc.compile()
    return nc


def _get_nc():
    global _compiled_nc
    if _compiled_nc is None:
        _compiled_nc = _build()
    return _compiled_nc


def _prep_inputs(inputs):
    x = np.ascontiguousarray(np.asarray(inputs["x"], dtype=np.float32))
    w = np.ascontiguousarray(np.asarray(inputs["weight"], dtype=np.float32))
    b = np.ascontiguousarray(np.asarray(inputs["bias"], dtype=np.float32))

    # x tiles per batch group r: [p, it*BR + b] = x_r[b, it*128 + p]
    xts = []
    for r in range(R):
        xs = x[r * BR:(r + 1) * BR, :]                      # [BR, IN_F]
        xt = xs.T.reshape(KT, P, BR).transpose(1, 0, 2)     # [P, KT, BR]
        xts.append(np.ascontiguousarray(
            xt.astype(_BF16_NP).reshape(P, KT * BR)))

    # W tiles per out-feature group c:
    # [ot*128 + p, it*128 + o2] = w_c[ot*128 + o2, it*128 + p]
    wts, bs = [], []
    for c in range(C):
        ws = w[c * OC:(c + 1) * OC, :]                      # [OC, IN_F]
        wtt = ws.T.reshape(KT, P, OT, P).transpose(2, 1, 0, 3)  # [OT,P,KT,P]
        wts.append(np.ascontiguousarray(
            wtt.astype(_BF16_NP).reshape(OT * P, KT * P)))
        bs.append(np.ascontiguousarray(b[c * OC:(c + 1) * OC].reshape(OT, P).T))

    in_maps = []
    for core in range(N_CORES):
        r, c = divmod(core, C)
        in_maps.append({"xt": xts[r], "wt": wts[c], "bias": bs[c]})
    return in_maps


def _run(inputs, trace=False, trace_cores=None):
    nc = _get_nc()
    in_maps = _prep_inputs(inputs)
    res = run_bass_kernel_spmd(nc, in_maps, core_ids=list(range(N_CORES)),
                               trace=trace, trace_cores=trace_cores)
    y = np.empty((BATCH, OUT_F), dtype=np.float32)
    for core in range(N_CORES):
        r, c = divmod(core, C)
        y[r * BR:(r + 1) * BR, c * OC:(c + 1) * OC] = res.results[core]["out"].T
    return y, res


def kernel(**inputs):
    y, _ = _run(inputs)
    return y


# revision 18
# speedup vs baseline: 1.1943x; 1.1943x over previous
"""Trainium2 Bass kernel for y = x @ W^T + b  (4096x4096 @ 4096x4096 + 4096).

Sharding: 2D grid, R=4 batch-groups x C=2 out-feature-groups. Core (r, c)
computes yT_rc = W_c @ x_r^T + b_c[:, None]  ([2048, 1024], output
transposed) and the host reassembles y. No collectives.

All layout work happens on the host: x and W slices are transposed,
tiled to the exact SBUF layout, and cast to bf16 in numpy. The device
kernel is nothing but back-to-back bf16 matmuls (fp32 PSUM accumulate):

  - xT_r [128, 32*1024] bf16 resident in SBUF, DMA'd in graduated
    chunks (small first so compute starts early).
  - Prologue: the first 2 o-tiles' accumulations run chunk-major over
    4 PSUM banks so the PE saturates while x is still arriving.
  - Steady state per o-tile: W slab [128, 32*128] bf16 DMA (triple-
    buffered), 32 k-tiles x 2 batch-chunk matmuls (N=512) accumulating
    in PSUM, ScalarE eviction fused with bias add, HWDGE DMA out.

PE roofline: 1024 MM x 512 cols / 2.4 GHz = 218.5 us per core.
"""

import os
import sys

for _p in ("/opt/trn_rl_repo", "/opt/pypackages"):
    if _p not in sys.path and os.path.isdir(_p):
        sys.path.append(_p)

import numpy as np
import ml_dtypes

import concourse.bass as bass
import concourse.tile as tile
from concourse import bacc, mybir
from concourse.bass_utils import run_bass_kernel_spmd

N_CORES = 8
R = 4                          # batch groups
C = 2                          # out-feature groups
BATCH = 4096
IN_F = 4096
OUT_F = 4096
P = 128
BR = BATCH // R                # 1024 batch rows per core
OC = OUT_F // C                # 2048 out features per core
KT = IN_F // P                 # 32 contraction tiles
OT = OC // P                   # 16 output-feature tiles per core
NB = BR // 512                 # 2 psum-width batch chunks
# x chunk schedule: (start_it, n_its); first chunks small so the PE can
# start sooner, later chunks 1MB for DMA efficiency.
XCHUNKS = [(0, 1), (1, 1), (2, 2), (4, 4), (8, 4), (12, 4), (16, 4),
           (20, 4), (24, 4), (28, 4)]
NPRE = 2                       # o-tiles interleaved in the prologue
WSPLIT = 8                     # k-tiles in the first piece of prologue W slabs

_F32 = mybir.dt.float32
_BF16 = mybir.dt.bfloat16
_BF16_NP = ml_dtypes.bfloat16

_compiled_nc = None


def _build():
    nc = bacc.Bacc("TRN2", target_bir_lowering=False, debug=False,
                   num_devices=N_CORES)

    # Host-pretiled layouts (see _prep_inputs):
    #   xt[p, it*BR + b]            = x_r[b, it*128 + p]              (bf16)
    #   wt[ot*128 + p, it*128 + o2] = w_c[ot*128 + o2, it*128 + p]    (bf16)
    #   bias_t[p, ot]               = b_c[ot*128 + p]                 (f32)
    xt = nc.dram_tensor("xt", [P, KT * BR], _BF16, kind="ExternalInput")
    wt = nc.dram_tensor("wt", [OT * P, KT * P], _BF16, kind="ExternalInput")
    bias = nc.dram_tensor("bias", [P, OT], _F32, kind="ExternalInput")
    out = nc.dram_tensor("out", [OC, BR], _F32, kind="ExternalOutput")

    with tile.TileContext(nc) as tc:
        with tc.tile_pool(name="const", bufs=1) as const, \
             tc.tile_pool(name="wslab", bufs=3) as wpool, \
             tc.tile_pool(name="psum", bufs=6, space="PSUM") as pspool, \
             tc.tile_pool(name="yout", bufs=3) as ypool:

            bias_sb = const.tile([P, OT], _F32)
            nc.scalar.dma_start(out=bias_sb[:], in_=bias[:, :])

            # ---- W slabs for the prologue o-tiles
            w_pre = []
            for ot in range(NPRE):
                w_sb = wpool.tile([P, KT * P], _BF16, name=f"w{ot}", tag="w")
                nc.sync.dma_start(out=w_sb[:],
                                  in_=wt[ot * P:(ot + 1) * P, :])
                w_pre.append(w_sb)

            # ---- x on the gpsimd (SWDGE) ring
            x_sb = const.tile([P, KT * BR], _BF16)
            for it0, nit in XCHUNKS:
                nc.gpsimd.dma_start(
                    out=x_sb[:, it0 * BR:(it0 + nit) * BR],
                    in_=xt[:, it0 * BR:(it0 + nit) * BR])

            def evict(ot, y_sb, ps, j):
                c0 = j * 512
                nc.scalar.activation(
                    y_sb[:, c0:c0 + 512], ps[:],
                    mybir.ActivationFunctionType.Identity,
                    bias=bias_sb[:, ot:ot + 1])
                nc.scalar.dma_start(
                    out=out[ot * P:(ot + 1) * P, c0:c0 + 512],
                    in_=y_sb[:, c0:c0 + 512])

            # ---- prologue: first NPRE o-tiles chunk-major over x arrival
            ps_pre = [[pspool.tile([P, 512], _F32, name=f"psp{ot}_{j}",
                                   tag="ps")
                       for j in range(NB)] for ot in range(NPRE)]
            for it0, nit in XCHUNKS:
                for ot in range(NPRE):
                    for l in range(nit):
                        it = it0 + l
                        for j in range(NB):
                            b0 = it * BR + j * 512
                            nc.tensor.matmul(
                                ps_pre[ot][j][:],
                                lhsT=w_pre[ot][:, it * P:(it + 1) * P],
                                rhs=x_sb[:, b0:b0 + 512],
                                start=(it == 0), stop=(it == KT - 1))
            for ot in range(NPRE):
                y_sb = ypool.tile([P, BR], _F32, name=f"y{ot}", tag="y")
                for j in range(NB):
                    evict(ot, y_sb, ps_pre[ot][j], j)

            # ---- steady state over the remaining o-tiles
            for ot in range(NPRE, OT):
                w_sb = wpool.tile([P, KT * P], _BF16, name=f"w{ot}", tag="w")
                nc.sync.dma_start(out=w_sb[:],
                                  in_=wt[ot * P:(ot + 1) * P, :])

                y_sb = ypool.tile([P, BR], _F32, name=f"y{ot}", tag="y")
                for j in range(NB):
                    ps = pspool.tile([P, 512], _F32, name=f"ps{ot}_{j}",
                                     tag="ps")
                    for it in range(KT):
                        b0 = it * BR + j * 512
                        nc.tensor.matmul(ps[:],
                                         lhsT=w_sb[:, it * P:(it + 1) * P],
                                         rhs=x_sb[:, b0:b0 + 512],
                                         start=(it == 0), stop=(it == KT - 1))
                    evict(ot, y_sb, ps, j)

    nc.compile()
    return nc


def _get_nc():
    global _compiled_nc
    if _compiled_nc is None:
        _compiled_nc = _build()
    return _compiled_nc


def _prep_inputs(inputs):
    x = np.ascontiguousarray(np.asarray(inputs["x"], dtype=np.float32))
    w = np.ascontiguousarray(np.asarray(inputs["weight"], dtype=np.float32))
    b = np.ascontiguousarray(np.asarray(inputs["bias"], dtype=np.float32))

    # x tiles per batch group r: [p, it*BR + b] = x_r[b, it*128 + p]
    xts = []
    for r in range(R):
        xs = x[r * BR:(r + 1) * BR, :]                      # [BR, IN_F]
        xt = xs.T.reshape(KT, P, BR).transpose(1, 0, 2)     # [P, KT, BR]
        xts.append(np.ascontiguousarray(
            xt.astype(_BF16_NP).reshape(P, KT * BR)))

    # W tiles per out-feature group c:
    # [ot*128 + p, it*128 + o2] = w_c[ot*128 + o2, it*128 + p]
    wts, bs = [], []
    for c in range(C):
        ws = w[c * OC:(c + 1) * OC, :]                      # [OC, IN_F]
        wtt = ws.T.reshape(KT, P, OT, P).transpose(2, 1, 0, 3)  # [OT,P,KT,P]
        wts.append(np.ascontiguousarray(
            wtt.astype(_BF16_NP).reshape(OT * P, KT * P)))
        bs.append(np.ascontiguousarray(b[c * OC:(c + 1) * OC].reshape(OT, P).T))

    in_maps = []
    for core in range(N_CORES):
        r, c = divmod(core, C)
        in_maps.append({"xt": xts[r], "wt": wts[c], "bias": bs[c]})
    return in_maps


def _run(inputs, trace=False, trace_cores=None):
    nc = _get_nc()
    in_maps = _prep_inputs(inputs)
    res = run_bass_kernel_spmd(nc, in_maps, core_ids=list(range(N_CORES)),
                               trace=trace, trace_cores=trace_cores)
    y = np.empty((BATCH, OUT_F), dtype=np.float32)
    for core in range(N_CORES):
        r, c = divmod(core, C)
        y[r * BR:(r + 1) * BR, c * OC:(c + 1) * OC] = res.results[core]["out"].T
    return y, res


def kernel(**inputs):
    y, _ = _run(inputs)
    return y
